# revision 14
# baseline (speedup 1.0000x reference)
"""Distributed kNN novelty-score kernel for Trainium2 (8 NeuronCores).

Problem: emb_state (256, 512), memory (200000, 512), K=5.
  d2[q, n] = ||q||^2 + ||m_n||^2 - 2 q.m_n
  score = mean over (q, k) of sqrt(d2 of the 5 nearest memory rows)

Strategy (memory rows sharded 8 ways, 25000 rows/core), fp8 edition:
  - Rank by v[q, n] = 2 q.m_n - (||m_n||^2 - 512). Data term in fp8e4m3
    with MatmulPerfMode.DoubleRow (2 fp8 weights/PE cell, 256-deep
    contraction per matmul): 2 DR matmuls cover D=512. The -r bias term
    (r = ||m||^2 - 512, hi/lo fp8 split, |r| < 240) is a K=2 matmul
    against a [2, 128] ones weight, so only 2 rows/chunk of bias are
    DMA'd (the fp16 baseline carried a full 128-row k-tile: 25% of DMA).
  - Chunks of 512 columns; blocks of 3 chunks share stationary weights
    (amortizes LDWEIGHTS, which DoubleRow can't fast-load). Per block the
    qt=0 banks finish first so DVE max8 (one call per 3 banks, straight
    from PSUM) overlaps the qt=1 matmuls.
  - Candidate exchange is split: a warm-up AllGather at t=0, a mid-stream
    AllGather for chunks 0..23 (latency hidden under compute), and a
    final AllGather for chunks 24..48. Merge: max8 over gathered 64
    values/query per segment, then over the 2x8 segment winners.
  - dist = sqrt((||q||^2 + 512) - v) on ACT; mean via ones-matmul
    partition reduction. Core 0's scalar is the answer.
"""

import sys

sys.path.insert(0, "/opt/trn_rl_repo")

import numpy as np

Q = 256
D = 512
N = 200000
K = 5
NCORES = 8
NSH = N // NCORES        # 25000 memory rows per core
P = 128
QT = Q // P              # 2 query tiles
FD = 512                 # free-dim chunk (one fp32 PSUM bank)
NCH = 49                 # chunks
NSHP = NCH * FD          # 25088 (padded shard length)
NG = 2                   # DoubleRow groups over D (2 x 256)
C_OFF = 512.0            # mean ||m||^2 folded into the sqrt bias
BLOCKS = [1, 2] + [3] * 15 + [1]          # chunks per PSUM block (sum 49)
SEG_SPLIT = 9                             # blocks 0..8 = seg0 (chunks 0..23)
G_SIZES = (1, 2, 3, 6, 9, 9, 9, 9, 1)     # chunks per DMA group (sum 49)
GMAX = max(G_SIZES)
NBLK = len(BLOCKS)
PAD_BIAS = -240.0        # per-row pad bias (v_pad = -480, never in top-5)

assert sum(BLOCKS) == NCH and sum(G_SIZES) == NCH

_CACHE = {}


def _build_bass():
    import concourse.bacc as bacc
    import concourse.mybir as mybir
    import concourse.tile as tile

    f32 = mybir.dt.float32
    f8 = mybir.dt.float8e4
    X = mybir.AxisListType.X
    DR = mybir.MatmulPerfMode.DoubleRowSwInterleave

    nc = bacc.Bacc(num_devices=NCORES)
    # [p, ch, g, i, f] = mem[ch*FD+f, g*256 + i*128 + p]
    mem8 = nc.declare_dram_parameter("mem8", [P, NCH, NG, 2, FD], f8, isOutput=False)
    # [2, ch, f]: hi/lo fp8 split of -(||m||^2 - 512)
    bias8 = nc.declare_dram_parameter("bias8", [2, NCH, FD], f8, isOutput=False)
    # [p, g, qt, i, m] = 2*emb[qt*128+m, g*256 + i*128 + p]
    wq8 = nc.declare_dram_parameter("wq8", [P, NG, QT, 2, P], f8, isOutput=False)
    onesb = nc.declare_dram_parameter("onesb", [2, P], f8, isOutput=False)
    sqq2 = nc.declare_dram_parameter("sqq2", [Q, 1], f32, isOutput=False)
    out = nc.declare_dram_parameter("out", [1, 1], f32, isOutput=True)

    with tile.TileContext(nc) as tc:
        with (
            tc.tile_pool(name="const", bufs=1) as cpool,
            tc.tile_pool(name="stream", bufs=3) as spool,
            tc.tile_pool(name="top", bufs=1) as tpool,
            tc.tile_pool(name="small", bufs=2) as mpool,
            tc.tile_pool(name="acc", bufs=2, space="PSUM") as ppool,
            tc.tile_pool(name="fin", bufs=1, space="PSUM") as fpool,
            tc.tile_pool(name="dram", bufs=1, space="DRAM") as dpool,
        ):
            # ---- constants ----
            wq_sb = cpool.tile([P, NG, QT, 2, P], f8)
            nc.sync.dma_start(out=wq_sb[:], in_=wq8[:, :, :, :, :])
            onesb_sb = cpool.tile([2, P], f8)
            nc.sync.dma_start(out=onesb_sb[:], in_=onesb[:, :])
            sqq_sb = cpool.tile([P, QT], f32)
            ones128 = cpool.tile([P, 1], f32)
            nc.vector.memset(ones128[:], 1.0)

            # ---- warm-up collective (pays CC start cost early) ----
            wup_sb = mpool.tile([1, 1], f32, tag="wup")
            nc.vector.memset(wup_sb[:], 0.0)
            wup_in = dpool.tile([1, 1], f32)
            nc.sync.dma_start(out=wup_in[:], in_=wup_sb[:])
            wup_out = dpool.tile([NCORES, 1], f32, addr_space="Shared")
            nc.gpsimd.collective_compute(
                "AllGather",
                mybir.AluOpType.bypass,
                replica_groups=[list(range(NCORES))],
                ins=[wup_in[:].opt()],
                outs=[wup_out[:].opt()],
            )

            # per-block top-8 candidates for every (query, q-tile)
            cand = tpool.tile([P, QT, NBLK, 8], f32)
            # per-segment merged global candidates
            m16 = tpool.tile([P, QT, 2, 8], f32)

            loc0 = dpool.tile([QT, P, K], f32)
            loc1 = dpool.tile([QT, P, K], f32)
            allc0 = dpool.tile([NCORES, QT, P, K], f32, addr_space="Shared")
            allc1 = dpool.tile([NCORES, QT, P, K], f32, addr_space="Shared")
            loc = [loc0, loc1]
            allc = [allc0, allc1]

            def flush_segment(seg, blk_lo, blk_hi):
                """local top-5 over blocks [blk_lo, blk_hi) -> AllGather."""
                l8 = mpool.tile([P, QT, 8], f32, tag="l8")
                for qt in range(QT):
                    nc.vector.max(l8[:, qt, :], cand[:, qt, blk_lo:blk_hi, :])
                nc.sync.dma_start(
                    out=loc[seg][:].rearrange("qt p k -> p qt k"),
                    in_=l8[:, :, 0:K],
                )
                nc.gpsimd.collective_compute(
                    "AllGather",
                    mybir.AluOpType.bypass,
                    replica_groups=[list(range(NCORES))],
                    ins=[loc[seg][:].opt()],
                    outs=[allc[seg][:].opt()],
                )
                # merge the 8 cores' candidates -> m16[:, qt, seg, :]
                gg = mpool.tile([P, QT, NCORES, K], f32, tag="gg")
                for qt in range(QT):
                    nc.sync.dma_start(
                        out=gg[:, qt, :, :],
                        in_=allc[seg][:, qt, :, :].rearrange("c p k -> p c k"),
                    )
                for qt in range(QT):
                    nc.vector.max(m16[:, qt, seg, :], gg[:, qt, :, :])

            # ---- main stream ----
            chunk_slot = []  # chunk -> (mem tile, bias tile, local idx)
            issued_blocks = 0
            chunks_ready = 0
            ch0 = 0

            def issue_blocks():
                nonlocal issued_blocks
                while (
                    issued_blocks < NBLK
                    and chunks_ready
                    >= sum(BLOCKS[: issued_blocks + 1])
                ):
                    b = issued_blocks
                    c_start = sum(BLOCKS[:b])
                    bsz = BLOCKS[b]
                    for qt in range(QT):
                        pt = ppool.tile([P, 3, FD], f32, tag="acc")
                        for c in range(bsz):
                            # K=2 bias matmul opens the accumulation group
                            _, bt, ci = chunk_slot[c_start + c]
                            nc.tensor.matmul(
                                pt[:, c, :],
                                onesb_sb[:],
                                bt[:, ci, :],
                                start=True,
                                stop=False,
                            )
                        for g in range(NG):
                            for c in range(bsz):
                                mt, _, ci = chunk_slot[c_start + c]
                                nc.tensor.matmul(
                                    pt[:, c, :],
                                    wq_sb[:, g, qt, :, :],
                                    mt[:, ci, g, :, :],
                                    start=False,
                                    stop=(g == NG - 1),
                                    perf_mode=DR,
                                )
                        nc.vector.max(
                            cand[:, qt, b, :],
                            pt[:, 0:bsz, :],
                        )
                    issued_blocks += 1
                    if issued_blocks == SEG_SPLIT:
                        flush_segment(0, 0, SEG_SPLIT)

            for gsz in G_SIZES:
                mt = spool.tile([P, GMAX, NG, 2, FD], f8, tag="memtile")
                nc.sync.dma_start(
                    out=mt[:, 0:gsz, :, :, :], in_=mem8[:, ch0 : ch0 + gsz, :, :, :]
                )
                bt = spool.tile([2, GMAX, FD], f8, tag="biastile")
                nc.sync.dma_start(
                    out=bt[:, 0:gsz, :], in_=bias8[:, ch0 : ch0 + gsz, :]
                )
                for c in range(gsz):
                    chunk_slot.append((mt, bt, c))
                chunks_ready += gsz
                ch0 += gsz
                issue_blocks()

            assert issued_blocks == NBLK
            flush_segment(1, SEG_SPLIT, NBLK)

            # ---- global top-5 and score ----
            nc.sync.dma_start(
                out=sqq_sb[:],
                in_=sqq2[:, :].rearrange("(qt p) one -> p (qt one)", p=P),
            )
            red = tpool.tile([P, QT], f32)
            for qt in range(QT):
                gfin = mpool.tile([P, 8], f32, tag="gfin")
                nc.vector.max(gfin[:], m16[:, qt, :, :])
                dist = mpool.tile([P, K], f32, tag="dist")
                # dist = sqrt(-v + (||q||^2 + 512)) = sqrt(d2)
                nc.scalar.activation(
                    dist[:],
                    gfin[:, 0:K],
                    mybir.ActivationFunctionType.Sqrt,
                    bias=sqq_sb[:, qt : qt + 1],
                    scale=-1.0,
                )
                nc.vector.reduce_sum(red[:, qt : qt + 1], dist[:], axis=X)

            pfin = fpool.tile([1, QT], f32)
            nc.tensor.matmul(pfin[:], ones128[:], red[:], start=True, stop=True)
            fin = mpool.tile([1, 1], f32, tag="fin")
            nc.vector.reduce_sum(fin[:], pfin[:], axis=X)
            nc.scalar.mul(fin[:], fin[:], 1.0 / (Q * K))
            nc.sync.dma_start(out=out[:, :], in_=fin[:])

    nc.compile()
    return nc


def _get_bass():
    if "nc" not in _CACHE:
        _CACHE["nc"] = _build_bass()
    return _CACHE["nc"]


def _to_fp8(x):
    import ml_dtypes

    return np.clip(x, -240.0, 240.0).astype(ml_dtypes.float8_e4m3fn)


def make_in_maps(emb_state: np.ndarray, memory: np.ndarray):
    """Shard + lay out inputs for the 8 cores."""
    import ml_dtypes

    emb_state = np.asarray(emb_state, dtype=np.float32)
    memory = np.asarray(memory, dtype=np.float32)

    # weights for DoubleRowSwInterleave: logical A[p, m] = 2*emb[qt*128+m,
    # g*256+p] (k-half 0), B (k-half 1); stored stream = A127,B127,A126,...
    embT2 = (2.0 * emb_state).T                       # [D, Q]
    base = embT2.reshape(NG, 2, P, QT, P).transpose(2, 0, 3, 1, 4)
    A = base[:, :, :, 0, ::-1]                        # [p, g, qt, m] reversed
    B = base[:, :, :, 1, ::-1]
    wq8 = _to_fp8(
        np.stack([A, B], axis=-1).reshape(P, NG, QT, 2, P)
    )
    onesb = np.ones((2, P), dtype=ml_dtypes.float8_e4m3fn)
    sqq2 = (np.sum(emb_state * emb_state, axis=1) + C_OFF).reshape(Q, 1)
    sqq2 = sqq2.astype(np.float32)

    in_maps = []
    for c in range(NCORES):
        m = memory[c * NSH : (c + 1) * NSH]                    # [25000, 512]
        mp = np.zeros((NSHP, D), dtype=np.float32)
        mp[:NSH] = m
        # mem8[p, ch, g, i, f] = mp[ch*FD+f, g*256 + i*128 + p]
        mem8 = _to_fp8(
            mp.reshape(NCH, FD, NG, 2, P).transpose(4, 0, 2, 3, 1)
        )
        # bias rows: -(||m||^2 - 512), padded rows -> -30000 (clips to -240/-240)
        negr = np.full(NSHP, -30000.0, dtype=np.float32)
        negr[:NSH] = -(
            np.sum(m.astype(np.float64) * m, axis=1).astype(np.float32) - C_OFF
        )
        hi = _to_fp8(negr)
        lo = _to_fp8(negr - hi.astype(np.float32))
        bias8 = np.stack([hi, lo], axis=0).reshape(2, NCH, FD)
        in_maps.append(
            {
                "mem8": mem8,
                "bias8": bias8,
                "wq8": wq8,
                "onesb": onesb,
                "sqq2": sqq2.copy(),
            }
        )
    return in_maps


def _install_ntff_hook():
    """Register the axon NTFF profile hook that this container's antenv lacks."""
    import sys as _sys
    import types

    if "antenv.axon_hooks" in _sys.modules:
        return
    try:
        import antenv
        from trn_agent_boot.trn_boot import _ntff_profile_via_ctypes

        hook = _ntff_profile_via_ctypes("/opt/axon/libaxon_pjrt.so")
        mod = types.ModuleType("antenv.axon_hooks")
        mod.get_axon_ntff_profile_hook = lambda: hook
        mod.set_axon_ntff_profile_hook = lambda h: None
        _sys.modules["antenv.axon_hooks"] = mod
        antenv.axon_hooks = mod
    except Exception as e:  # profiling is best-effort
        print(f"ntff hook install failed: {e}")


def _run(in_maps, trace=False):
    from concourse.bass_utils import run_bass_kernel_spmd

    if trace:
        _install_ntff_hook()
    nc = _get_bass()
    res = run_bass_kernel_spmd(
        nc, in_maps, core_ids=list(range(NCORES)), trace=trace
    )
    return res


def kernel(emb_state: np.ndarray, memory: np.ndarray) -> np.ndarray:
    in_maps = make_in_maps(emb_state, memory)
    res = _run(in_maps, trace=False)
    val = np.float32(res.results[0]["out"].reshape(-1)[0])
    return np.asarray(val, dtype=np.float32).reshape(())


# revision 15
# speedup vs baseline: 1.0790x; 1.0790x over previous
"""Distributed kNN novelty-score kernel for Trainium2 (8 NeuronCores).

Problem: emb_state (256, 512), memory (200000, 512), K=5.
  d2[q, n] = ||q||^2 + ||m_n||^2 - 2 q.m_n
  score = mean over (q, k) of sqrt(d2 of the 5 nearest memory rows)

Strategy (memory rows sharded 8 ways, 25000 rows/core), fp8 edition:
  - Rank by v[q, n] = 2 q.m_n - (||m_n||^2 - 512). Data term in fp8e4m3
    with MatmulPerfMode.DoubleRow (2 fp8 weights/PE cell, 256-deep
    contraction per matmul): 2 DR matmuls cover D=512. The -r bias term
    (r = ||m||^2 - 512, hi/lo fp8 split, |r| < 240) is a K=2 matmul
    against a [2, 128] ones weight, so only 2 rows/chunk of bias are
    DMA'd (the fp16 baseline carried a full 128-row k-tile: 25% of DMA).
  - Chunks of 512 columns; blocks of 3 chunks share stationary weights
    (amortizes LDWEIGHTS, which DoubleRow can't fast-load). Per block the
    qt=0 banks finish first so DVE max8 (one call per 3 banks, straight
    from PSUM) overlaps the qt=1 matmuls.
  - Candidate exchange is split: a warm-up AllGather at t=0, a mid-stream
    AllGather for chunks 0..23 (latency hidden under compute), and a
    final AllGather for chunks 24..48. Merge: max8 over gathered 64
    values/query per segment, then over the 2x8 segment winners.
  - dist = sqrt((||q||^2 + 512) - v) on ACT; mean via ones-matmul
    partition reduction. Core 0's scalar is the answer.
"""

import sys

sys.path.insert(0, "/opt/trn_rl_repo")

import numpy as np

Q = 256
D = 512
N = 200000
K = 5
NCORES = 8
NSH = N // NCORES        # 25000 memory rows per core
P = 128
QT = Q // P              # 2 query tiles
FD = 512                 # free-dim chunk (one fp32 PSUM bank)
NCH = 49                 # chunks
NSHP = NCH * FD          # 25088 (padded shard length)
NG = 2                   # DoubleRow groups over D (2 x 256)
C_OFF = 512.0            # mean ||m||^2 folded into the sqrt bias
BLOCKS = [1, 2] + [3] * 15 + [1]          # chunks per PSUM block (sum 49)
SEG_SPLIT = 9                             # blocks 0..8 = seg0 (chunks 0..23)
G_SIZES = (1, 2, 3, 6, 9, 9, 9, 9, 1)     # chunks per DMA group (sum 49)
GMAX = max(G_SIZES)
NBLK = len(BLOCKS)
PAD_BIAS = -240.0        # per-row pad bias (v_pad = -480, never in top-5)

assert sum(BLOCKS) == NCH and sum(G_SIZES) == NCH

_CACHE = {}


def _build_bass():
    import concourse.bacc as bacc
    import concourse.mybir as mybir
    import concourse.tile as tile

    f32 = mybir.dt.float32
    f8 = mybir.dt.float8e4
    X = mybir.AxisListType.X
    DR = mybir.MatmulPerfMode.DoubleRow

    nc = bacc.Bacc(num_devices=NCORES)
    # [p, ch, g, i, f] = mem[ch*FD+f, g*256 + i*128 + p]
    mem8 = nc.declare_dram_parameter("mem8", [P, NCH, NG, 2, FD], f8, isOutput=False)
    # [2, ch, f]: hi/lo fp8 split of -(||m||^2 - 512)
    bias8 = nc.declare_dram_parameter("bias8", [2, NCH, FD], f8, isOutput=False)
    # [p, g, qt, i, m] = 2*emb[qt*128+m, g*256 + i*128 + p]
    wq8 = nc.declare_dram_parameter("wq8", [P, NG, QT, 2, P], f8, isOutput=False)
    onesb = nc.declare_dram_parameter("onesb", [2, P], f8, isOutput=False)
    sqq2 = nc.declare_dram_parameter("sqq2", [Q, 1], f32, isOutput=False)
    out = nc.declare_dram_parameter("out", [1, 1], f32, isOutput=True)

    with tile.TileContext(nc) as tc:
        with (
            tc.tile_pool(name="const", bufs=1) as cpool,
            tc.tile_pool(name="stream", bufs=3) as spool,
            tc.tile_pool(name="top", bufs=1) as tpool,
            tc.tile_pool(name="small", bufs=2) as mpool,
            tc.tile_pool(name="acc", bufs=2, space="PSUM") as ppool,
            tc.tile_pool(name="fin", bufs=1, space="PSUM") as fpool,
            tc.tile_pool(name="dram", bufs=1, space="DRAM") as dpool,
        ):
            # ---- constants ----
            wq_sb = cpool.tile([P, NG, QT, 2, P], f8)
            nc.sync.dma_start(out=wq_sb[:], in_=wq8[:, :, :, :, :])
            onesb_sb = cpool.tile([2, P], f8)
            nc.sync.dma_start(out=onesb_sb[:], in_=onesb[:, :])
            sqq_sb = cpool.tile([P, QT], f32)
            ones128 = cpool.tile([P, 1], f32)
            nc.vector.memset(ones128[:], 1.0)

            # ---- warm-up collective (pays CC start cost early) ----
            wup_sb = mpool.tile([1, 1], f32, tag="wup")
            nc.vector.memset(wup_sb[:], 0.0)
            wup_in = dpool.tile([1, 1], f32)
            nc.sync.dma_start(out=wup_in[:], in_=wup_sb[:])
            wup_out = dpool.tile([NCORES, 1], f32, addr_space="Shared")
            nc.gpsimd.collective_compute(
                "AllGather",
                mybir.AluOpType.bypass,
                replica_groups=[list(range(NCORES))],
                ins=[wup_in[:].opt()],
                outs=[wup_out[:].opt()],
            )

            # per-block top-8 candidates for every (query, q-tile)
            cand = tpool.tile([P, QT, NBLK, 8], f32)
            # per-segment merged global candidates
            m16 = tpool.tile([P, QT, 2, 8], f32)

            loc0 = dpool.tile([QT, P, K], f32)
            loc1 = dpool.tile([QT, P, K], f32)
            allc0 = dpool.tile([NCORES, QT, P, K], f32, addr_space="Shared")
            allc1 = dpool.tile([NCORES, QT, P, K], f32, addr_space="Shared")
            loc = [loc0, loc1]
            allc = [allc0, allc1]

            def flush_segment(seg, blk_lo, blk_hi):
                """local top-5 over blocks [blk_lo, blk_hi) -> AllGather."""
                l8 = mpool.tile([P, QT, 8], f32, tag="l8")
                for qt in range(QT):
                    nc.vector.max(l8[:, qt, :], cand[:, qt, blk_lo:blk_hi, :])
                nc.sync.dma_start(
                    out=loc[seg][:].rearrange("qt p k -> p qt k"),
                    in_=l8[:, :, 0:K],
                )
                nc.gpsimd.collective_compute(
                    "AllGather",
                    mybir.AluOpType.bypass,
                    replica_groups=[list(range(NCORES))],
                    ins=[loc[seg][:].opt()],
                    outs=[allc[seg][:].opt()],
                )
                # merge the 8 cores' candidates -> m16[:, qt, seg, :]
                gg = mpool.tile([P, QT, NCORES, K], f32, tag="gg")
                for qt in range(QT):
                    nc.sync.dma_start(
                        out=gg[:, qt, :, :],
                        in_=allc[seg][:, qt, :, :].rearrange("c p k -> p c k"),
                    )
                for qt in range(QT):
                    nc.vector.max(m16[:, qt, seg, :], gg[:, qt, :, :])

            # ---- main stream ----
            chunk_slot = []  # chunk -> (mem tile, bias tile, local idx)
            issued_blocks = 0
            chunks_ready = 0
            ch0 = 0

            def issue_blocks():
                nonlocal issued_blocks
                while (
                    issued_blocks < NBLK
                    and chunks_ready
                    >= sum(BLOCKS[: issued_blocks + 1])
                ):
                    b = issued_blocks
                    c_start = sum(BLOCKS[:b])
                    bsz = BLOCKS[b]
                    for qt in range(QT):
                        pt = ppool.tile([P, 3, FD], f32, tag="acc")
                        for c in range(bsz):
                            # K=2 bias matmul opens the accumulation group
                            _, bt, ci = chunk_slot[c_start + c]
                            nc.tensor.matmul(
                                pt[:, c, :],
                                onesb_sb[:],
                                bt[:, ci, :],
                                start=True,
                                stop=False,
                            )
                        for g in range(NG):
                            for c in range(bsz):
                                mt, _, ci = chunk_slot[c_start + c]
                                nc.tensor.matmul(
                                    pt[:, c, :],
                                    wq_sb[:, g, qt, :, :],
                                    mt[:, ci, g, :, :],
                                    start=False,
                                    stop=(g == NG - 1),
                                    perf_mode=DR,
                                )
                        nc.vector.max(
                            cand[:, qt, b, :],
                            pt[:, 0:bsz, :],
                        )
                    issued_blocks += 1
                    if issued_blocks == SEG_SPLIT:
                        flush_segment(0, 0, SEG_SPLIT)

            for gsz in G_SIZES:
                mt = spool.tile([P, GMAX, NG, 2, FD], f8, tag="memtile")
                nc.sync.dma_start(
                    out=mt[:, 0:gsz, :, :, :], in_=mem8[:, ch0 : ch0 + gsz, :, :, :]
                )
                bt = spool.tile([2, GMAX, FD], f8, tag="biastile")
                nc.sync.dma_start(
                    out=bt[:, 0:gsz, :], in_=bias8[:, ch0 : ch0 + gsz, :]
                )
                for c in range(gsz):
                    chunk_slot.append((mt, bt, c))
                chunks_ready += gsz
                ch0 += gsz
                issue_blocks()

            assert issued_blocks == NBLK
            flush_segment(1, SEG_SPLIT, NBLK)

            # ---- global top-5 and score ----
            nc.sync.dma_start(
                out=sqq_sb[:],
                in_=sqq2[:, :].rearrange("(qt p) one -> p (qt one)", p=P),
            )
            red = tpool.tile([P, QT], f32)
            for qt in range(QT):
                gfin = mpool.tile([P, 8], f32, tag="gfin")
                nc.vector.max(gfin[:], m16[:, qt, :, :])
                dist = mpool.tile([P, K], f32, tag="dist")
                # dist = sqrt(-v + (||q||^2 + 512)) = sqrt(d2)
                nc.scalar.activation(
                    dist[:],
                    gfin[:, 0:K],
                    mybir.ActivationFunctionType.Sqrt,
                    bias=sqq_sb[:, qt : qt + 1],
                    scale=-1.0,
                )
                nc.vector.reduce_sum(red[:, qt : qt + 1], dist[:], axis=X)

            pfin = fpool.tile([1, QT], f32)
            nc.tensor.matmul(pfin[:], ones128[:], red[:], start=True, stop=True)
            fin = mpool.tile([1, 1], f32, tag="fin")
            nc.vector.reduce_sum(fin[:], pfin[:], axis=X)
            nc.scalar.mul(fin[:], fin[:], 1.0 / (Q * K))
            nc.sync.dma_start(out=out[:, :], in_=fin[:])

    nc.compile()
    return nc


def _get_bass():
    if "nc" not in _CACHE:
        _CACHE["nc"] = _build_bass()
    return _CACHE["nc"]


def _to_fp8(x):
    import ml_dtypes

    return np.clip(x, -240.0, 240.0).astype(ml_dtypes.float8_e4m3fn)


def make_in_maps(emb_state: np.ndarray, memory: np.ndarray):
    """Shard + lay out inputs for the 8 cores."""
    import ml_dtypes

    emb_state = np.asarray(emb_state, dtype=np.float32)
    memory = np.asarray(memory, dtype=np.float32)

    # weights: [p, g, qt, i, m] = 2*emb[qt*128+m, g*256+i*128+p]
    embT2 = (2.0 * emb_state).T                       # [D, Q]
    wq8 = _to_fp8(
        embT2.reshape(NG, 2, P, QT, P).transpose(2, 0, 3, 1, 4)
    )
    onesb = np.ones((2, P), dtype=ml_dtypes.float8_e4m3fn)
    sqq2 = (np.sum(emb_state * emb_state, axis=1) + C_OFF).reshape(Q, 1)
    sqq2 = sqq2.astype(np.float32)

    in_maps = []
    for c in range(NCORES):
        m = memory[c * NSH : (c + 1) * NSH]                    # [25000, 512]
        mp = np.zeros((NSHP, D), dtype=np.float32)
        mp[:NSH] = m
        # mem8[p, ch, g, i, f] = mp[ch*FD+f, g*256 + i*128 + p]
        mem8 = _to_fp8(
            mp.reshape(NCH, FD, NG, 2, P).transpose(4, 0, 2, 3, 1)
        )
        # bias rows: -(||m||^2 - 512), padded rows -> -30000 (clips to -240/-240)
        negr = np.full(NSHP, -30000.0, dtype=np.float32)
        negr[:NSH] = -(
            np.sum(m.astype(np.float64) * m, axis=1).astype(np.float32) - C_OFF
        )
        hi = _to_fp8(negr)
        lo = _to_fp8(negr - hi.astype(np.float32))
        bias8 = np.stack([hi, lo], axis=0).reshape(2, NCH, FD)
        in_maps.append(
            {
                "mem8": mem8,
                "bias8": bias8,
                "wq8": wq8,
                "onesb": onesb,
                "sqq2": sqq2.copy(),
            }
        )
    return in_maps


def _install_ntff_hook():
    """Register the axon NTFF profile hook that this container's antenv lacks."""
    import sys as _sys
    import types

    if "antenv.axon_hooks" in _sys.modules:
        return
    try:
        import antenv
        from trn_agent_boot.trn_boot import _ntff_profile_via_ctypes

        hook = _ntff_profile_via_ctypes("/opt/axon/libaxon_pjrt.so")
        mod = types.ModuleType("antenv.axon_hooks")
        mod.get_axon_ntff_profile_hook = lambda: hook
        mod.set_axon_ntff_profile_hook = lambda h: None
        _sys.modules["antenv.axon_hooks"] = mod
        antenv.axon_hooks = mod
    except Exception as e:  # profiling is best-effort
        print(f"ntff hook install failed: {e}")


def _run(in_maps, trace=False):
    from concourse.bass_utils import run_bass_kernel_spmd

    if trace:
        _install_ntff_hook()
    nc = _get_bass()
    res = run_bass_kernel_spmd(
        nc, in_maps, core_ids=list(range(NCORES)), trace=trace
    )
    return res


def kernel(emb_state: np.ndarray, memory: np.ndarray) -> np.ndarray:
    in_maps = make_in_maps(emb_state, memory)
    res = _run(in_maps, trace=False)
    val = np.float32(res.results[0]["out"].reshape(-1)[0])
    return np.asarray(val, dtype=np.float32).reshape(())


# revision 20
# speedup vs baseline: 1.0874x; 1.0077x over previous
"""Distributed kNN novelty-score kernel for Trainium2 (8 NeuronCores).

Problem: emb_state (256, 512), memory (200000, 512), K=5.
  d2[q, n] = ||q||^2 + ||m_n||^2 - 2 q.m_n
  score = mean over (q, k) of sqrt(d2 of the 5 nearest memory rows)

Strategy (memory rows sharded 8 ways, 25000 rows/core), fp8 edition:
  - Rank by v[q, n] = 2 q.m_n - (||m_n||^2 - 512). Data term in fp8e4m3
    with MatmulPerfMode.DoubleRow (2 fp8 weights/PE cell, 256-deep
    contraction per matmul): 2 DR matmuls cover D=512. The -r bias term
    (r = ||m||^2 - 512, hi/lo fp8 split, |r| < 240) is a K=2 matmul
    against a [2, 128] ones weight, so only 2 rows/chunk of bias are
    DMA'd (the fp16 baseline carried a full 128-row k-tile: 25% of DMA).
  - Chunks of 512 columns; blocks of 3 chunks share stationary weights
    (amortizes LDWEIGHTS, which DoubleRow can't fast-load). Per block the
    qt=0 banks finish first so DVE max8 (one call per 3 banks, straight
    from PSUM) overlaps the qt=1 matmuls.
  - Candidate exchange is split: a warm-up AllGather at t=0, a mid-stream
    AllGather for chunks 0..23 (latency hidden under compute), and a
    final AllGather for chunks 24..48. Merge: max8 over gathered 64
    values/query per segment, then over the 2x8 segment winners.
  - dist = sqrt((||q||^2 + 512) - v) on ACT; mean via ones-matmul
    partition reduction. Core 0's scalar is the answer.
"""

import sys

sys.path.insert(0, "/opt/trn_rl_repo")

import numpy as np

Q = 256
D = 512
N = 200000
K = 5
NCORES = 8
NSH = N // NCORES        # 25000 memory rows per core
P = 128
QT = Q // P              # 2 query tiles
FD = 512                 # free-dim chunk (one fp32 PSUM bank)
NCH = 49                 # chunks
NSHP = NCH * FD          # 25088 (padded shard length)
NG = 2                   # DoubleRow groups over D (2 x 256)
C_OFF = 512.0            # mean ||m||^2 folded into the sqrt bias
BLOCKS = [1, 2] + [3] * 15 + [1]          # chunks per PSUM block (sum 49)
SEG_SPLIT = 9                             # blocks 0..8 = seg0 (chunks 0..23)
G_SIZES = (1, 2, 3, 6, 8, 8, 8, 8, 5)     # chunks per DMA group (sum 49)
GMAX = max(G_SIZES)
NBLK = len(BLOCKS)
PAD_BIAS = -240.0        # per-row pad bias (v_pad = -480, never in top-5)

assert sum(BLOCKS) == NCH and sum(G_SIZES) == NCH

_CACHE = {}


def _build_bass():
    import concourse.bacc as bacc
    import concourse.mybir as mybir
    import concourse.tile as tile

    f32 = mybir.dt.float32
    f8 = mybir.dt.float8e4
    X = mybir.AxisListType.X
    DR = mybir.MatmulPerfMode.DoubleRow

    nc = bacc.Bacc(num_devices=NCORES)
    # [p, ch, g, i, f] = mem[ch*FD+f, g*256 + i*128 + p]
    mem8 = nc.declare_dram_parameter("mem8", [P, NCH, NG, 2, FD], f8, isOutput=False)
    # [2, ch, f]: hi/lo fp8 split of -(||m||^2 - 512)
    bias8 = nc.declare_dram_parameter("bias8", [2, NCH, FD], f8, isOutput=False)
    # [p, g, qt, i, m] = 2*emb[qt*128+m, g*256 + i*128 + p]
    wq8 = nc.declare_dram_parameter("wq8", [P, NG, QT, 2, P], f8, isOutput=False)
    onesb = nc.declare_dram_parameter("onesb", [2, P], f8, isOutput=False)
    sqq2 = nc.declare_dram_parameter("sqq2", [Q, 1], f32, isOutput=False)
    out = nc.declare_dram_parameter("out", [1, 1], f32, isOutput=True)

    with tile.TileContext(nc) as tc:
        with (
            tc.tile_pool(name="const", bufs=1) as cpool,
            tc.tile_pool(name="stream", bufs=4) as spool,
            tc.tile_pool(name="top", bufs=1) as tpool,
            tc.tile_pool(name="small", bufs=2) as mpool,
            tc.tile_pool(name="acc", bufs=2, space="PSUM") as ppool,
            tc.tile_pool(name="fin", bufs=1, space="PSUM") as fpool,
            tc.tile_pool(name="dram", bufs=1, space="DRAM") as dpool,
        ):
            # ---- constants ----
            wq_sb = cpool.tile([P, NG, QT, 2, P], f8)
            nc.sync.dma_start(out=wq_sb[:], in_=wq8[:, :, :, :, :])
            onesb_sb = cpool.tile([2, P], f8)
            nc.sync.dma_start(out=onesb_sb[:], in_=onesb[:, :])
            sqq_sb = cpool.tile([P, QT], f32)
            ones128 = cpool.tile([P, 1], f32)
            nc.vector.memset(ones128[:], 1.0)

            # per-block top-8 candidates for every (query, q-tile)
            cand = tpool.tile([P, QT, NBLK, 8], f32)
            # per-segment merged global candidates
            m16 = tpool.tile([P, QT, 2, 8], f32)

            loc0 = dpool.tile([QT, P, K], f32)
            loc1 = dpool.tile([QT, P, K], f32)
            allc0 = dpool.tile([NCORES, QT, P, K], f32, addr_space="Shared")
            allc1 = dpool.tile([NCORES, QT, P, K], f32, addr_space="Shared")
            loc = [loc0, loc1]
            allc = [allc0, allc1]

            def flush_segment(seg, blk_lo, blk_hi):
                """local top-5 over blocks [blk_lo, blk_hi) -> AllGather."""
                l8 = mpool.tile([P, QT, 8], f32, tag="l8")
                for qt in range(QT):
                    nc.vector.max(l8[:, qt, :], cand[:, qt, blk_lo:blk_hi, :])
                nc.sync.dma_start(
                    out=loc[seg][:].rearrange("qt p k -> p qt k"),
                    in_=l8[:, :, 0:K],
                )
                nc.gpsimd.collective_compute(
                    "AllGather",
                    mybir.AluOpType.bypass,
                    replica_groups=[list(range(NCORES))],
                    ins=[loc[seg][:].opt()],
                    outs=[allc[seg][:].opt()],
                )
                # merge the 8 cores' candidates -> m16[:, qt, seg, :]
                gg = mpool.tile([P, QT, NCORES, K], f32, tag="gg")
                for qt in range(QT):
                    nc.sync.dma_start(
                        out=gg[:, qt, :, :],
                        in_=allc[seg][:, qt, :, :].rearrange("c p k -> p c k"),
                    )
                for qt in range(QT):
                    nc.vector.max(m16[:, qt, seg, :], gg[:, qt, :, :])

            # ---- main stream ----
            chunk_slot = []  # chunk -> (mem tile, bias tile, local idx)
            issued_blocks = 0
            chunks_ready = 0
            ch0 = 0

            def issue_blocks():
                nonlocal issued_blocks
                while (
                    issued_blocks < NBLK
                    and chunks_ready
                    >= sum(BLOCKS[: issued_blocks + 1])
                ):
                    b = issued_blocks
                    c_start = sum(BLOCKS[:b])
                    bsz = BLOCKS[b]
                    for qt in range(QT):
                        pt = ppool.tile([P, 3, FD], f32, tag="acc")
                        for c in range(bsz):
                            # K=2 bias matmul opens the accumulation group
                            _, bt, ci = chunk_slot[c_start + c]
                            nc.tensor.matmul(
                                pt[:, c, :],
                                onesb_sb[:],
                                bt[:, ci, :],
                                start=True,
                                stop=False,
                            )
                        for g in range(NG):
                            for c in range(bsz):
                                mt, _, ci = chunk_slot[c_start + c]
                                nc.tensor.matmul(
                                    pt[:, c, :],
                                    wq_sb[:, g, qt, :, :],
                                    mt[:, ci, g, :, :],
                                    start=False,
                                    stop=(g == NG - 1),
                                    perf_mode=DR,
                                )
                        nc.vector.max(
                            cand[:, qt, b, :],
                            pt[:, 0:bsz, :],
                        )
                    issued_blocks += 1
                    if issued_blocks == SEG_SPLIT:
                        flush_segment(0, 0, SEG_SPLIT)

            for gsz in G_SIZES:
                mt = spool.tile([P, GMAX, NG, 2, FD], f8, tag="memtile")
                nc.sync.dma_start(
                    out=mt[:, 0:gsz, :, :, :], in_=mem8[:, ch0 : ch0 + gsz, :, :, :]
                )
                bt = spool.tile([2, GMAX, FD], f8, tag="biastile")
                nc.sync.dma_start(
                    out=bt[:, 0:gsz, :], in_=bias8[:, ch0 : ch0 + gsz, :]
                )
                for c in range(gsz):
                    chunk_slot.append((mt, bt, c))
                chunks_ready += gsz
                ch0 += gsz
                issue_blocks()

            assert issued_blocks == NBLK
            flush_segment(1, SEG_SPLIT, NBLK)

            # ---- global top-5 and score ----
            nc.sync.dma_start(
                out=sqq_sb[:],
                in_=sqq2[:, :].rearrange("(qt p) one -> p (qt one)", p=P),
            )
            dist = tpool.tile([P, QT * K], f32)
            for qt in range(QT):
                gfin = mpool.tile([P, 8], f32, tag="gfin")
                nc.vector.max(gfin[:], m16[:, qt, :, :])
                # dist = sqrt(-v + (||q||^2 + 512)) = sqrt(d2)
                nc.scalar.activation(
                    dist[:, qt * K : (qt + 1) * K],
                    gfin[:, 0:K],
                    mybir.ActivationFunctionType.Sqrt,
                    bias=sqq_sb[:, qt : qt + 1],
                    scale=-1.0,
                )
            red = tpool.tile([P, 1], f32)
            nc.vector.reduce_sum(red[:], dist[:, :], axis=X)
            pfin = fpool.tile([1, 1], f32)
            nc.tensor.matmul(pfin[:], ones128[:], red[:], start=True, stop=True)
            fin = mpool.tile([1, 1], f32, tag="fin")
            nc.scalar.activation(
                fin[:],
                pfin[:],
                mybir.ActivationFunctionType.Copy,
                scale=1.0 / (Q * K),
            )
            nc.sync.dma_start(out=out[:, :], in_=fin[:])

    nc.compile()
    return nc


def _get_bass():
    if "nc" not in _CACHE:
        _CACHE["nc"] = _build_bass()
    return _CACHE["nc"]


def _to_fp8(x):
    import ml_dtypes

    return np.clip(x, -240.0, 240.0).astype(ml_dtypes.float8_e4m3fn)


def make_in_maps(emb_state: np.ndarray, memory: np.ndarray):
    """Shard + lay out inputs for the 8 cores."""
    import ml_dtypes

    emb_state = np.asarray(emb_state, dtype=np.float32)
    memory = np.asarray(memory, dtype=np.float32)

    # weights: [p, g, qt, i, m] = 2*emb[qt*128+m, g*256+i*128+p]
    embT2 = (2.0 * emb_state).T                       # [D, Q]
    wq8 = _to_fp8(
        embT2.reshape(NG, 2, P, QT, P).transpose(2, 0, 3, 1, 4)
    )
    onesb = np.ones((2, P), dtype=ml_dtypes.float8_e4m3fn)
    sqq2 = (np.sum(emb_state * emb_state, axis=1) + C_OFF).reshape(Q, 1)
    sqq2 = sqq2.astype(np.float32)

    in_maps = []
    for c in range(NCORES):
        m = memory[c * NSH : (c + 1) * NSH]                    # [25000, 512]
        mp = np.zeros((NSHP, D), dtype=np.float32)
        mp[:NSH] = m
        # mem8[p, ch, g, i, f] = mp[ch*FD+f, g*256 + i*128 + p]
        mem8 = _to_fp8(
            mp.reshape(NCH, FD, NG, 2, P).transpose(4, 0, 2, 3, 1)
        )
        # bias rows: -(||m||^2 - 512), padded rows -> -30000 (clips to -240/-240)
        negr = np.full(NSHP, -30000.0, dtype=np.float32)
        negr[:NSH] = -(
            np.sum(m.astype(np.float64) * m, axis=1).astype(np.float32) - C_OFF
        )
        hi = _to_fp8(negr)
        lo = _to_fp8(negr - hi.astype(np.float32))
        bias8 = np.stack([hi, lo], axis=0).reshape(2, NCH, FD)
        in_maps.append(
            {
                "mem8": mem8,
                "bias8": bias8,
                "wq8": wq8,
                "onesb": onesb,
                "sqq2": sqq2.copy(),
            }
        )
    return in_maps


def _install_ntff_hook():
    """Register the axon NTFF profile hook that this container's antenv lacks."""
    import sys as _sys
    import types

    if "antenv.axon_hooks" in _sys.modules:
        return
    try:
        import antenv
        from trn_agent_boot.trn_boot import _ntff_profile_via_ctypes

        hook = _ntff_profile_via_ctypes("/opt/axon/libaxon_pjrt.so")
        mod = types.ModuleType("antenv.axon_hooks")
        mod.get_axon_ntff_profile_hook = lambda: hook
        mod.set_axon_ntff_profile_hook = lambda h: None
        _sys.modules["antenv.axon_hooks"] = mod
        antenv.axon_hooks = mod
    except Exception as e:  # profiling is best-effort
        print(f"ntff hook install failed: {e}")


def _run(in_maps, trace=False):
    from concourse.bass_utils import run_bass_kernel_spmd

    if trace:
        _install_ntff_hook()
    nc = _get_bass()
    res = run_bass_kernel_spmd(
        nc, in_maps, core_ids=list(range(NCORES)), trace=trace
    )
    return res


def kernel(emb_state: np.ndarray, memory: np.ndarray) -> np.ndarray:
    in_maps = make_in_maps(emb_state, memory)
    res = _run(in_maps, trace=False)
    val = np.float32(res.results[0]["out"].reshape(-1)[0])
    return np.asarray(val, dtype=np.float32).reshape(())
